# revision 1
# baseline (speedup 1.0000x reference)
"""Trainium2 Bass kernel for nn_CNNVectorForm (LeNet-style CNN, batch 8192).

Pipeline per core (data-parallel over batch, 1024 images/core):
  conv 5x5 VALID (1->20ch, 28->24)  -> 2x2 maxpool -> fc1(2880->500) + relu
  -> fc2(500->10) + softmax

Device formulation:
  * All activations feature-major [features, batch] so the PE contracts
    along partitions; batch rides the free dim (512 per tile).
  * Conv as a Toeplitz matmul: for each output row r and 12-wide column
    block, one K=80 (5 rows x 16 cols of input) x M=120 matmul produces
    [20ch x 12cols, batch].  Output columns are split into even/odd
    M-tiles so the 2x2 maxpool is three partition-aligned tensor_max ops.
  * fc1 weights are host-permuted to the pooled-feature order, so fc1 is
    24 accumulating K=120 matmuls per 125-neuron M-tile.
  * conv bias is folded into the fc1 bias on the host (maxpool commutes
    with the per-channel constant).
  * fc2 runs batch-major (stationary operand = activations) so softmax
    reduces along the free dim; fc2 bias via a K=1 ones matmul.
  * Matmuls use float32r (1 cycle/row at N>=256 vs 4 for fp32).
"""

import numpy as np

N, H, W = 8192, 28, 28
COUT, KS = 20, 5
NCORES = 8
NPC = N // NCORES  # images per core
CONV_W_OUT = 24
PH = 12            # pooled rows
FC1_IN, FC1_OUT, FC2_OUT = 2880, 500, 10
MT, MTS = 4, 125   # fc1 M tiles
KB, KBS = 24, 120  # a1 feature blocks (one per (pooled row, column half))

_cache = {}


def _build(npc, nb):
    from contextlib import ExitStack

    import concourse.tile as tile
    from concourse import bacc, mybir

    f32 = mybir.dt.float32
    f32r = mybir.dt.float32r
    nbt = npc // nb

    nc = bacc.Bacc(
        "TRN2",
        target_bir_lowering=False,
        debug=False,
        enable_asserts=False,
        num_devices=NCORES,
    )

    # host-im2col'd input: xg[jb, p, r, b] = x[(r + p//16)*28 + 12*jb + p%16, b]
    xg_d = nc.dram_tensor(
        "xg", [2, 80, CONV_W_OUT, npc], f32r, kind="ExternalInput"
    ).ap()
    t_d = nc.dram_tensor("tmat", [80, 240], f32r, kind="ExternalInput").ap()
    w1_d = nc.dram_tensor(
        "w1", [KB // 4, KBS, 4 * FC1_OUT], f32r, kind="ExternalInput"
    ).ap()
    b1_d = nc.dram_tensor("b1", [MTS, MT], f32, kind="ExternalInput").ap()
    w2_d = nc.dram_tensor("w2", [MTS, MT * FC2_OUT], f32r, kind="ExternalInput").ap()
    b2_d = nc.dram_tensor("b2", [FC2_OUT, 1], f32, kind="ExternalInput").ap()
    o_d = nc.dram_tensor("out", [npc, FC2_OUT], f32, kind="ExternalOutput").ap()

    with tile.TileContext(nc) as tc, ExitStack() as ctx:
        const = ctx.enter_context(tc.tile_pool(name="const", bufs=1))
        w1pool = ctx.enter_context(tc.tile_pool(name="w1", bufs=6))
        gpool = ctx.enter_context(tc.tile_pool(name="gather", bufs=8))
        a1pool = ctx.enter_context(tc.tile_pool(name="a1", bufs=8))
        tmppool = ctx.enter_context(tc.tile_pool(name="ptmp", bufs=4))
        a2pool = ctx.enter_context(tc.tile_pool(name="a2", bufs=2 * MT))
        smpool = ctx.enter_context(tc.tile_pool(name="softmax", bufs=4))
        cpsum = ctx.enter_context(tc.tile_pool(name="cpsum", bufs=4, space="PSUM"))
        fpsum = ctx.enter_context(tc.tile_pool(name="fpsum", bufs=4, space="PSUM"))

        from concourse.masks import make_identity

        t240 = const.tile([80, 240], f32r)
        nc.sync.dma_start(t240[:], t_d[:])
        # fc1 weights: 6 grouped DMAs of 4 blocks each, host-packed so every
        # group is one fully-contiguous [120, 2000] transfer.  Issued from
        # the compute-free gpsimd engine: issuing these from sync or scalar
        # blocks the gathers / pool-eviction copies behind the weight stream
        # and stalls the whole conv pipeline.
        WG = 4
        w1g = []
        for gidx in range(KB // WG):
            wt = w1pool.tile([KBS, WG * FC1_OUT], f32r, tag="w1",
                             name=f"w1g{gidx}")
            nc.gpsimd.dma_start(wt[:], w1_d[gidx])
            w1g.append(wt)
        b1t = const.tile([MTS, MT], f32)
        nc.scalar.dma_start(b1t[:], b1_d[:])
        w2t = const.tile([MTS, MT * FC2_OUT], f32r)
        nc.scalar.dma_start(w2t[:], w2_d[:])
        b2t = const.tile([FC2_OUT, 1], f32)
        nc.scalar.dma_start(b2t[:], b2_d[:])
        ident = const.tile([FC2_OUT, FC2_OUT], f32)
        make_identity(nc, ident[:])

        def w1_slice(j, mt):
            return w1g[j // WG][
                :, (j % WG) * FC1_OUT + mt * MTS : (j % WG) * FC1_OUT + (mt + 1) * MTS
            ]

        for bt in range(nbt):
            b0 = bt * nb
            a1 = [None] * KB
            # fc1 accumulators for all 4 M-tiles ride along with the conv
            # loop, skewed by 2 blocks: 4 dependency-free fc1 matmuls per
            # quad keep the PE gap-free so HAM stays at full clock.
            fp = [
                fpsum.tile([MTS, nb], f32, tag="fps", name=f"fp{bt}_{mt}")
                for mt in range(MT)
            ]
            SKEW = 4
            for kb in range(KB + SKEW):
                if kb >= SKEW:
                    j = kb - SKEW
                    for mt in range(MT):
                        nc.tensor.matmul(
                            fp[mt][:],
                            w1_slice(j, mt),
                            a1[j][:],
                            start=(j == 0),
                            stop=(j == KB - 1),
                        )
                if kb >= KB:
                    continue
                ip, jb = kb // 2, kb % 2
                g = []
                for dr in range(2):
                    gt = gpool.tile([80, nb], f32r, tag="g")
                    r = 2 * ip + dr
                    nc.sync.dma_start(gt[:], xg_d[jb, :, r, b0 : b0 + nb])
                    g.append(gt)
                ps = [
                    cpsum.tile([KBS, nb], f32, tag="cps", name=f"cps{i}")
                    for i in range(4)
                ]
                for dr in range(2):
                    for eo in range(2):
                        nc.tensor.matmul(
                            ps[2 * dr + eo][:],
                            t240[:, 120 * eo : 120 * (eo + 1)],
                            g[dr][:],
                            start=True,
                            stop=True,
                        )
                s0 = tmppool.tile([KBS, nb], f32, tag="s")
                nc.scalar.copy(s0[:], ps[0][:])
                m0 = tmppool.tile([KBS, nb], f32, tag="m")
                nc.vector.tensor_max(m0[:], s0[:], ps[1][:])
                s1 = tmppool.tile([KBS, nb], f32, tag="s")
                nc.scalar.copy(s1[:], ps[2][:])
                m1 = tmppool.tile([KBS, nb], f32, tag="m")
                nc.vector.tensor_max(m1[:], s1[:], ps[3][:])
                ab = a1pool.tile([KBS, nb], f32r, tag="a1")
                nc.vector.tensor_max(ab[:], m0[:], m1[:])
                a1[kb] = ab

            a2t = [None] * MT
            for mt in range(MT):
                a2 = a2pool.tile([MTS, nb], f32r, tag="a2")
                nc.scalar.activation(
                    a2[:],
                    fp[mt][:],
                    mybir.ActivationFunctionType.Relu,
                    bias=b1t[:, mt : mt + 1],
                )
                a2t[mt] = a2

            # fc2 feature-major: weights stationary, batch streams; softmax
            # needs batch on partitions, so PE-transpose 128-wide slices.
            p2f = fpsum.tile([FC2_OUT, nb], f32, tag="fps", name=f"p2f_{bt}")
            for mt in range(MT):
                nc.tensor.matmul(
                    p2f[:],
                    w2t[:, mt * FC2_OUT : (mt + 1) * FC2_OUT],
                    a2t[mt][:],
                    start=(mt == 0),
                    stop=(mt == MT - 1),
                )
            s2 = smpool.tile([FC2_OUT, nb], f32, tag="s2")
            nc.scalar.activation(
                s2[:], p2f[:], mybir.ActivationFunctionType.Identity,
                bias=b2t[:, 0:1],
            )
            sub = min(128, nb)
            for s in range(nb // sub):
                tp = fpsum.tile([sub, FC2_OUT], f32, tag="fps",
                                name=f"tp_{bt}_{s}")
                nc.tensor.transpose(
                    tp[:], s2[:, s * sub : (s + 1) * sub], ident[:]
                )
                e = smpool.tile([sub, FC2_OUT], f32, tag="e")
                ssum = smpool.tile([sub, 1], f32, tag="ss")
                nc.scalar.activation(
                    e[:], tp[:], mybir.ActivationFunctionType.Exp,
                    accum_out=ssum[:],
                )
                rinv = smpool.tile([sub, 1], f32, tag="ri")
                nc.vector.reciprocal(rinv[:], ssum[:])
                ot = smpool.tile([sub, FC2_OUT], f32, tag="ot")
                nc.vector.tensor_scalar_mul(ot[:], e[:], rinv[:])
                nc.sync.dma_start(o_d[b0 + s * sub : b0 + (s + 1) * sub, :], ot[:])

    nc.compile()
    return nc


def _prep_weights(conv_w, conv_b, fc1_w, fc1_b, fc2_w, fc2_b):
    conv_w = np.asarray(conv_w, np.float32).reshape(COUT, KS, KS)
    conv_b = np.asarray(conv_b, np.float32)
    fc1_w = np.asarray(fc1_w, np.float32)
    fc1_b = np.asarray(fc1_b, np.float32)
    fc2_w = np.asarray(fc2_w, np.float32)
    fc2_b = np.asarray(fc2_b, np.float32)

    # Toeplitz conv matrix [80, 240]: row = di*16 + jjp (input row offset,
    # input col within 16-wide block); col m = eo*120 + c*6 + q for output
    # col jj = 2q + eo within the 12-wide block.
    T = np.zeros((80, 240), np.float32)
    for m in range(240):
        eo, c, q = m // 120, (m % 120) // 6, m % 6
        jj = 2 * q + eo
        for di in range(KS):
            for dj in range(KS):
                T[di * 16 + jj + dj, m] = conv_w[c, di, dj]

    # fc1 weights permuted to our pooled-feature order:
    # block kb = ip*2 + jb, within-block m = c*6 + q
    # -> original flat feature c*144 + ip*12 + jb*6 + q
    kbv = np.arange(KB)
    ipv, jbv = kbv // 2, kbv % 2
    ml = np.arange(KBS)
    cv, qv = ml // 6, ml % 6
    fidx = cv[None, :] * 144 + ipv[:, None] * 12 + jbv[:, None] * 6 + qv[None, :]
    w1 = fc1_w.T[fidx.reshape(-1)].reshape(KB, KBS, FC1_OUT)
    # pack into 6 contiguous groups of 4 blocks: [6, 120, 4*500]
    w1 = np.ascontiguousarray(
        w1.reshape(KB // 4, 4, KBS, FC1_OUT).transpose(0, 2, 1, 3)
    ).reshape(KB // 4, KBS, 4 * FC1_OUT)

    # conv bias folded into fc1 bias (pool-max commutes with per-channel const)
    cb_vec = np.repeat(conv_b, 144)
    b1p = fc1_b + fc1_w @ cb_vec
    b1 = np.ascontiguousarray(b1p.reshape(MT, MTS).T)

    w2 = np.ascontiguousarray(
        fc2_w.T.reshape(MT, MTS, FC2_OUT).transpose(1, 0, 2)
    ).reshape(MTS, MT * FC2_OUT)
    b2 = np.ascontiguousarray(fc2_b.reshape(FC2_OUT, 1))
    return T, w1, b1, w2, b2


# im2col pixel indices: idx[jb, di*16+jjp, r] = (r+di)*28 + 12*jb + jjp
_IDX = np.zeros((2, 80, CONV_W_OUT), np.int64)
for _jb in range(2):
    for _di in range(KS):
        for _jjp in range(16):
            for _r in range(CONV_W_OUT):
                _IDX[_jb, _di * 16 + _jjp, _r] = (_r + _di) * W + 12 * _jb + _jjp


def _prep_x(x_core):
    """x_core [784, npc] pixel-major -> xg [2, 80, 24, npc]."""
    return np.ascontiguousarray(x_core[_IDX.reshape(-1)].reshape(
        2, 80, CONV_W_OUT, x_core.shape[1]))


def _run(inputs, npc=NPC, nb=512, trace=False):
    from concourse import bass_utils

    key = (npc, nb)
    if key not in _cache:
        _cache[key] = _build(npc, nb)
    nc = _cache[key]

    T, w1, b1, w2, b2 = _prep_weights(
        inputs["conv_w"], inputs["conv_b"], inputs["fc1_w"],
        inputs["fc1_b"], inputs["fc2_w"], inputs["fc2_b"],
    )
    x = np.asarray(inputs["x"], np.float32).reshape(-1, H * W)
    n_total = x.shape[0]
    assert n_total == NCORES * npc
    xs = x.reshape(NCORES, npc, H * W).transpose(0, 2, 1)

    in_maps = [
        {"xg": _prep_x(xs[i]), "tmat": T, "w1": w1, "b1": b1, "w2": w2,
         "b2": b2}
        for i in range(NCORES)
    ]
    res = bass_utils.run_bass_kernel_spmd(
        nc, in_maps, core_ids=list(range(NCORES)), trace=trace
    )
    out = np.concatenate([res.results[i]["out"] for i in range(NCORES)], axis=0)
    return out, res


def kernel(**inputs):
    out, _ = _run(inputs)
    return out



# revision 6
# speedup vs baseline: 1.4924x; 1.4924x over previous
"""Trainium2 Bass kernel for nn_CNNVectorForm (LeNet-style CNN, batch 8192).

Pipeline per core (data-parallel over batch, 1024 images/core):
  conv 5x5 VALID (1->20ch, 28->24) -> 2x2 maxpool -> fc1(2880->500) + relu
  -> fc2(500->10) + softmax

Device formulation (v2 — bf16, coalesced DMA, dense PE):
  * Everything feature-major [features, batch]; batch rides the free dim
    (nb=256 per tile, 4 tiles per core).
  * Inputs, conv/fc weights in bf16 (validated ~4.6e-3 final rel err);
    PSUM accumulation stays fp32.
  * Conv as Toeplitz matmul with K=128 = 8 input rows x 16 cols gathers:
    host im2col expansion is only 1.96x (3.1MB/core in bf16), loaded as
    6 big DMAs (128 descriptors each) instead of 96 small gathers - the
    baseline's DMA-issue serialization starved the PE below 1.2GHz.
  * One 8-row gather feeds 4 conv rows = 2 pooled-row "sets"; a set is
    4 matmuls (2 conv rows x 2 col-parities) into 2 PSUM bank tiles
    [120, 2, 256] packed so 2x2 maxpool is: scalar copy (PSUM->SBUF),
    DVE max (SBUF,PSUM), gpsimd max (SBUF halves) -> a1 bf16.
  * fc1 runs 1:1 interleaved with conv (4+4 matmuls per set, lag 3) so
    the PE never idles and HAM holds the 2.4GHz clock; accumulators for
    all 4 M-tiles live in one 2-bank PSUM tile [125, 4, 256].
  * conv bias folded into the fc1 bias on the host.
  * softmax feature-major: exp via scalar (bias=b2), row-sum via an
    all-ones [10,10] matmul, DVE reciprocal + multiply; output [10, npc]
    is transposed on the host.
"""

import numpy as np

N, H, W = 8192, 28, 28
COUT, KS = 20, 5
NCORES = 8
NPC = N // NCORES        # images per core
G = 6                    # row groups of 8 input rows (stride 4)
JB = 2                   # 16-wide column blocks at offsets 0, 12
NSEG = JB * G            # 12 (jb, g) segments -> 24 pooled-feature blocks
FC1_IN, FC1_OUT, FC2_OUT = 2880, 500, 10
MT, MTS = 4, 125         # fc1 M tiles
KB, KBS = 24, 120        # fc1 K blocks (one per (pooled row, column half))
LAG = 3                  # fc1 trails conv by this many sets

_cache = {}


def _build(npc, nb):
    from contextlib import ExitStack

    import concourse.tile as tile
    from concourse import bacc, mybir

    f32 = mybir.dt.float32
    bf16 = mybir.dt.bfloat16
    nbt = npc // nb

    nc = bacc.Bacc(
        "TRN2",
        target_bir_lowering=False,
        debug=False,
        enable_asserts=False,
        num_devices=NCORES,
    )

    # host-im2col'd input: xg[p, jb, g, b] = x[(4g + p//16)*28 + 12jb + p%16, b]
    xg_d = nc.dram_tensor("xg", [128, JB, G, npc], bf16, kind="ExternalInput").ap()
    t8_d = nc.dram_tensor("t8", [128, 8 * KBS], bf16, kind="ExternalInput").ap()
    w1_d = nc.dram_tensor("w1", [KBS, KB * FC1_OUT], bf16, kind="ExternalInput").ap()
    b1_d = nc.dram_tensor("b1", [MTS, MT], f32, kind="ExternalInput").ap()
    w2_d = nc.dram_tensor("w2", [MTS, MT * FC2_OUT], bf16, kind="ExternalInput").ap()
    b2_d = nc.dram_tensor("b2", [FC2_OUT, 1], f32, kind="ExternalInput").ap()
    o_d = nc.dram_tensor("out", [FC2_OUT, npc], f32, kind="ExternalOutput").ap()

    with tile.TileContext(nc) as tc, ExitStack() as ctx:
        const = ctx.enter_context(tc.tile_pool(name="const", bufs=1))
        spool = ctx.enter_context(tc.tile_pool(name="spool", bufs=3))
        mpool = ctx.enter_context(tc.tile_pool(name="mpool", bufs=3))
        a1pool = ctx.enter_context(tc.tile_pool(name="a1", bufs=6))
        a2pool = ctx.enter_context(tc.tile_pool(name="a2", bufs=8))
        smpool = ctx.enter_context(tc.tile_pool(name="softmax", bufs=2))
        cpsum = ctx.enter_context(tc.tile_pool(name="cpsum", bufs=4, space="PSUM"))
        fpsum = ctx.enter_context(tc.tile_pool(name="fpsum", bufs=1, space="PSUM"))
        tpsum = ctx.enter_context(tc.tile_pool(name="tpsum", bufs=1, space="PSUM"))

        # --- constants / weights -------------------------------------------
        # scalar ring: small tensors first, then the fc1 weight stream.
        t8s = const.tile([128, 8 * KBS], bf16)
        nc.scalar.dma_start(t8s[:], t8_d[:])
        b1t = const.tile([MTS, MT], f32)
        nc.scalar.dma_start(b1t[:], b1_d[:])
        w2t = const.tile([MTS, MT * FC2_OUT], bf16)
        nc.scalar.dma_start(w2t[:], w2_d[:])
        b2t = const.tile([FC2_OUT, 1], f32)
        nc.scalar.dma_start(b2t[:], b2_d[:])
        w1t = []
        for jc in range(4):
            wt = const.tile([KBS, 6 * FC1_OUT], bf16, name=f"w1c{jc}")
            nc.scalar.dma_start(wt[:], w1_d[:, jc * 6 * FC1_OUT : (jc + 1) * 6 * FC1_OUT])
            w1t.append(wt)
        # sync ring: the input stream, 2 segments (= 4KB/partition) per DMA.
        xpair = []
        for i in range(NSEG // 2):
            jb, gp = i // 3, i % 3
            xt = const.tile([128, 2 * npc], bf16, name=f"xp{i}")
            nc.sync.dma_start(xt[:], xg_d[:, jb, 2 * gp : 2 * gp + 2, :])
            xpair.append(xt)

        ones10 = const.tile([FC2_OUT, FC2_OUT], bf16)
        nc.gpsimd.memset(ones10[:], 1.0)
        # warm the scalar activation table set (exp+relu+copy) while idle
        wtile = const.tile([1, 8], f32)
        nc.gpsimd.memset(wtile[:], 0.0)
        wout = const.tile([1, 8], f32)
        nc.scalar.activation(wout[:], wtile[:], mybir.ActivationFunctionType.Exp)

        outbuf = const.tile([FC2_OUT, npc], f32)

        def w1_slice(j, mt):
            c0 = (j % 6) * FC1_OUT + mt * MTS
            return w1t[j // 6][:, c0 : c0 + MTS]

        def fc1_mms(fp, a1, j, b0):
            # fp = (fpA, fpB): two 1-bank tiles, each packing 2 M-tiles.
            # start/stop are bank-wide (has_written clears the whole bank),
            # so only the first/last matmul touching a bank carries the flag.
            for mt in range(MT):
                nc.tensor.matmul(
                    fp[mt // 2][:, mt % 2, :],
                    w1_slice(j, mt),
                    a1[j][:],
                    start=(j == 0 and mt % 2 == 0),
                    stop=(j == KB - 1 and mt % 2 == 1),
                )

        def tail(bt, fp, a1_unused, b0):
            # relu (+bias) -> fc2 -> softmax for batch tile bt
            a2t = []
            for mt in range(MT):
                a2 = a2pool.tile([MTS, nb], bf16, tag="a2", name=f"a2_{bt}_{mt}")
                nc.scalar.activation(
                    a2[:],
                    fp[mt // 2][:, mt % 2, :],
                    mybir.ActivationFunctionType.Relu,
                    bias=b1t[:, mt : mt + 1],
                )
                a2t.append(a2)
            p2f = tpsum.tile([FC2_OUT, nb], f32, tag="p2f", name=f"p2f_{bt}")
            for mt in range(MT):
                nc.tensor.matmul(
                    p2f[:],
                    w2t[:, mt * FC2_OUT : (mt + 1) * FC2_OUT],
                    a2t[mt][:],
                    start=(mt == 0),
                    stop=(mt == MT - 1),
                )
            e = smpool.tile([FC2_OUT, nb], bf16, tag="e", name=f"e_{bt}")
            nc.scalar.activation(
                e[:], p2f[:], mybir.ActivationFunctionType.Exp, bias=b2t[:, 0:1]
            )
            ssum = tpsum.tile([FC2_OUT, nb], f32, tag="ssum", name=f"ss_{bt}")
            nc.tensor.matmul(ssum[:], ones10[:], e[:], start=True, stop=True)
            rinv = smpool.tile([FC2_OUT, nb], f32, tag="ri", name=f"ri_{bt}")
            nc.vector.reciprocal(rinv[:], ssum[:])
            nc.vector.tensor_mul(outbuf[:, b0 : b0 + nb], e[:], rinv[:])
            nc.sync.dma_start(o_d[:, b0 : b0 + nb], outbuf[:, b0 : b0 + nb])

        pending_tail = None
        for bt in range(nbt):
            b0 = bt * nb
            a1 = [None] * KB
            fp = (
                fpsum.tile([MTS, 2, nb], f32, tag="fpA", name=f"fpA{bt}"),
                fpsum.tile([MTS, 2, nb], f32, tag="fpB", name=f"fpB{bt}"),
            )
            for idx in range(NSEG):
                rhs = xpair[idx // 2][:, (idx % 2) * npc + b0 : (idx % 2) * npc + b0 + nb]
                for st in range(2):
                    sidx = idx * 2 + st
                    # conv: 4 matmuls -> 2 one-bank PSUM tiles [120, 2, nb]
                    # t0 = conv row 4g+2st (both col parities), t1 = row +1
                    t0 = cpsum.tile([KBS, 2, nb], f32, tag="cps", name=f"t0_{bt}_{sidx}")
                    t1 = cpsum.tile([KBS, 2, nb], f32, tag="cps", name=f"t1_{bt}_{sidx}")
                    for half, tt in ((0, t0), (1, t1)):
                        dd = 2 * st + half
                        for eo in range(2):
                            nc.tensor.matmul(
                                tt[:, eo, :],
                                t8s[:, (dd * 2 + eo) * KBS : (dd * 2 + eo + 1) * KBS],
                                rhs,
                                start=(eo == 0),
                                stop=(eo == 1),
                            )
                    # 2x2 maxpool: max over (t0, t1) x (eo halves).  PSUM
                    # evacuation is split between scalar and DVE; every 4th
                    # set uses 2 scalar copies + a cheap bf16 DVE max instead
                    # of 1 copy + a slow fp32-PSUM DVE max, balancing the two
                    # engines at ~76us/core each (Pool TT is unsupported).
                    m = mpool.tile([KBS, 2, nb], bf16, tag="m")
                    if sidx % 4 == 3:
                        s0 = spool.tile([KBS, 2, nb], bf16, tag="s")
                        nc.scalar.copy(s0[:], t0[:])
                        s1 = spool.tile([KBS, 2, nb], bf16, tag="s")
                        nc.scalar.copy(s1[:], t1[:])
                        nc.vector.tensor_max(m[:], s0[:], s1[:])
                    else:
                        s = spool.tile([KBS, 2, nb], f32, tag="sf")
                        nc.scalar.copy(s[:], t0[:])
                        nc.vector.tensor_max(m[:], s[:], t1[:])
                    ab = a1pool.tile([KBS, nb], bf16, tag="a1")
                    nc.vector.tensor_max(ab[:], m[:, 0, :], m[:, 1, :])
                    a1[sidx] = ab
                    # fc1 trails conv so the PE alternates 4 conv + 4 fc1
                    if sidx >= LAG:
                        fc1_mms(fp, a1, sidx - LAG, b0)
                    if pending_tail is not None and sidx == 1:
                        tail(*pending_tail)
                        pending_tail = None
            for j in range(KB - LAG, KB):
                fc1_mms(fp, a1, j, b0)
            pending_tail = (bt, fp, a1, b0)
        tail(*pending_tail)

    nc.compile()
    return nc


def _prep_weights(conv_w, conv_b, fc1_w, fc1_b, fc2_w, fc2_b):
    import ml_dtypes

    bf16 = ml_dtypes.bfloat16
    conv_w = np.asarray(conv_w, np.float32).reshape(COUT, KS, KS)
    conv_b = np.asarray(conv_b, np.float32)
    fc1_w = np.asarray(fc1_w, np.float32)
    fc1_b = np.asarray(fc1_b, np.float32)
    fc2_w = np.asarray(fc2_w, np.float32)
    fc2_b = np.asarray(fc2_b, np.float32)

    # conv stationaries: T8[p = di8*16+jjp, k = dd*2+eo, m = c*6+q]
    #  = conv_w[c, di8-dd, jjp-(2q+eo)] where both offsets are in [0, 5)
    T8 = np.zeros((128, 8, KBS), np.float32)
    for dd in range(4):
        for eo in range(2):
            for c in range(COUT):
                for q in range(6):
                    jj = 2 * q + eo
                    for di in range(KS):
                        for dj in range(KS):
                            T8[(dd + di) * 16 + jj + dj, dd * 2 + eo, c * 6 + q] = (
                                conv_w[c, di, dj]
                            )
    t8 = np.ascontiguousarray(T8.reshape(128, 8 * KBS)).astype(bf16)

    # fc1 weights to pooled-feature order: block j=(jb*6+g)*2+st, row c*6+q
    # -> original flat feature c*144 + (2g+st)*12 + jb*6 + q
    jv = np.arange(KB)
    jbv, gv, stv = jv // 12, (jv % 12) // 2, jv % 2
    ipv = 2 * gv + stv
    ml = np.arange(KBS)
    cv, qv = ml // 6, ml % 6
    fidx = (
        cv[None, :] * 144 + ipv[:, None] * 12 + jbv[:, None] * 6 + qv[None, :]
    )  # [KB, KBS]
    w1 = fc1_w[:, fidx]                       # [500, KB, KBS]
    w1 = np.ascontiguousarray(w1.transpose(2, 1, 0)).reshape(KBS, KB * FC1_OUT)
    w1 = w1.astype(bf16)

    # conv bias folded into fc1 bias (maxpool commutes with per-channel const)
    cb_vec = np.repeat(conv_b, 144)
    b1p = fc1_b + fc1_w @ cb_vec
    b1 = np.ascontiguousarray(b1p.reshape(MT, MTS).T)

    w2 = np.ascontiguousarray(
        fc2_w.T.reshape(MT, MTS, FC2_OUT).transpose(1, 0, 2)
    ).reshape(MTS, MT * FC2_OUT).astype(bf16)
    b2 = np.ascontiguousarray(fc2_b.reshape(FC2_OUT, 1))
    return t8, w1, b1, w2, b2


# im2col pixel indices: IDX[p, jb, g] = (4g + p//16)*28 + 12jb + (p%16)
_P = np.arange(128)
_IDX = (
    (4 * np.arange(G)[None, None, :] + (_P // 16)[:, None, None]) * W
    + 12 * np.arange(JB)[None, :, None]
    + (_P % 16)[:, None, None]
)  # [128, JB, G]


def _prep_x(x_bf16):
    """x_bf16 [npc, 784] bf16 -> xg [128, JB, G, npc] bf16."""
    g = x_bf16[:, _IDX.reshape(-1)]           # [npc, 128*JB*G]
    g = g.T.reshape(128, JB, G, x_bf16.shape[0])
    return np.ascontiguousarray(g)


def _run(inputs, npc=NPC, nb=256, trace=False):
    import ml_dtypes
    from concourse import bass_utils

    key = (npc, nb)
    if key not in _cache:
        _cache[key] = _build(npc, nb)
    nc = _cache[key]

    t8, w1, b1, w2, b2 = _prep_weights(
        inputs["conv_w"], inputs["conv_b"], inputs["fc1_w"],
        inputs["fc1_b"], inputs["fc2_w"], inputs["fc2_b"],
    )
    x = np.asarray(inputs["x"], np.float32).reshape(-1, H * W)
    n_total = x.shape[0]
    assert n_total == NCORES * npc
    xb = x.astype(ml_dtypes.bfloat16)

    in_maps = [
        {
            "xg": _prep_x(xb[i * npc : (i + 1) * npc]),
            "t8": t8, "w1": w1, "b1": b1, "w2": w2, "b2": b2,
        }
        for i in range(NCORES)
    ]
    res = bass_utils.run_bass_kernel_spmd(
        nc, in_maps, core_ids=list(range(NCORES)), trace=trace
    )
    out = np.concatenate(
        [np.asarray(res.results[i]["out"], np.float32).T for i in range(NCORES)],
        axis=0,
    )
    return out, res


def kernel(**inputs):
    out, _ = _run(inputs)
    return out


# revision 7
# speedup vs baseline: 1.5172x; 1.0166x over previous
"""Trainium2 Bass kernel for nn_CNNVectorForm (LeNet-style CNN, batch 8192).

Pipeline per core (data-parallel over batch, 1024 images/core):
  conv 5x5 VALID (1->20ch, 28->24) -> 2x2 maxpool -> fc1(2880->500) + relu
  -> fc2(500->10) + softmax

Device formulation (v3 — bf16, coalesced DMA, dense+warm PE):
  * Everything feature-major [features, batch]; batch rides the free dim
    (nb=256 per tile, 4 tiles per core).
  * Inputs, conv/fc weights in bf16 (validated ~4.6e-3 final rel err);
    PSUM accumulation stays fp32.
  * Conv as Toeplitz matmul with K=128 = 8 input rows x 16 cols gathers:
    host im2col expansion is only 1.96x (3.1MB/core in bf16), loaded as
    13 contiguous per-partition DMAs (128 descriptors each) instead of
    96 small gathers - the baseline's DMA-issue serialization starved
    the PE below 1.2GHz.
  * One 8-row gather feeds 4 conv rows = 2 pooled-row "sets"; a set is
    4 matmuls (2 conv rows x 2 col-parities) into 2 one-bank PSUM tiles
    [120, 2, 256] packed so 2x2 maxpool is: scalar copy (PSUM->SBUF),
    DVE max (SBUF,PSUM), DVE bf16 max (halves) -> a1 bf16.  Every 6th
    set swaps work between scalar and DVE to balance both at ~95us.
  * fc1 runs 1:1 interleaved with conv (4+4 matmuls per set, lag 3) so
    the PE never idles; dummy warm-up matmuls during the ~7us framework
    preamble bring the PE HAM clock to 2.4GHz before real work starts.
  * fc1 accumulators: 2 one-bank PSUM tiles [125, 2, 256]; start/stop
    flags are bank-granular (has_written clears whole banks).
  * conv bias folded into the fc1 bias on the host.
  * The relu/fc2/softmax tail of each batch tile is staggered across
    the first sets of the next tile so it hides under conv/fc1; softmax
    row-sum via an all-ones [10,10] matmul, fast-approx reciprocal;
    output [10, npc] is transposed on the host.
  * DMA rings: sync (HWDGE) carries t8 + input; scalar carries the tiny
    biases/fc2; gpsimd (SWDGE) streams the fc1 weights.
"""

import numpy as np

N, H, W = 8192, 28, 28
COUT, KS = 20, 5
NCORES = 8
NPC = N // NCORES        # images per core
G = 6                    # row groups of 8 input rows (stride 4)
JB = 2                   # 16-wide column blocks at offsets 0, 12
NSEG = JB * G            # 12 (jb, g) segments -> 24 pooled-feature blocks
FC1_IN, FC1_OUT, FC2_OUT = 2880, 500, 10
MT, MTS = 4, 125         # fc1 M tiles
KB, KBS = 24, 120        # fc1 K blocks (one per (pooled row, column half))
LAG = 3                  # fc1 trails conv by this many sets
NWARM = 10               # PE warm-up matmuls

_cache = {}


def _build(npc, nb):
    from contextlib import ExitStack

    import concourse.tile as tile
    from concourse import bacc, mybir

    f32 = mybir.dt.float32
    bf16 = mybir.dt.bfloat16
    nbt = npc // nb

    nc = bacc.Bacc(
        "TRN2",
        target_bir_lowering=False,
        debug=False,
        enable_asserts=False,
        num_devices=NCORES,
    )

    # host-im2col'd input: xg[p, jb, g, b] = x[(4g + p//16)*28 + 12jb + p%16, b]
    xg_d = nc.dram_tensor("xg", [128, JB, G, npc], bf16, kind="ExternalInput").ap()
    t8_d = nc.dram_tensor("t8", [128, 8 * KBS], bf16, kind="ExternalInput").ap()
    w1_d = nc.dram_tensor("w1", [KBS, KB * FC1_OUT], bf16, kind="ExternalInput").ap()
    b1_d = nc.dram_tensor("b1", [MTS, MT], f32, kind="ExternalInput").ap()
    w2_d = nc.dram_tensor("w2", [MTS, MT * FC2_OUT], bf16, kind="ExternalInput").ap()
    b2_d = nc.dram_tensor("b2", [FC2_OUT, 1], f32, kind="ExternalInput").ap()
    o_d = nc.dram_tensor("out", [FC2_OUT, npc], f32, kind="ExternalOutput").ap()

    with tile.TileContext(nc) as tc, ExitStack() as ctx:
        const = ctx.enter_context(tc.tile_pool(name="const", bufs=1))
        spool = ctx.enter_context(tc.tile_pool(name="spool", bufs=3))
        mpool = ctx.enter_context(tc.tile_pool(name="mpool", bufs=3))
        a1pool = ctx.enter_context(tc.tile_pool(name="a1", bufs=6))
        a2pool = ctx.enter_context(tc.tile_pool(name="a2", bufs=8))
        smpool = ctx.enter_context(tc.tile_pool(name="softmax", bufs=2))
        cpsum = ctx.enter_context(tc.tile_pool(name="cpsum", bufs=4, space="PSUM"))
        fpsum = ctx.enter_context(tc.tile_pool(name="fpsum", bufs=1, space="PSUM"))
        tpsum = ctx.enter_context(tc.tile_pool(name="tpsum", bufs=1, space="PSUM"))

        # --- PE warm-up: HAM releases the 2.4GHz clock only after ~3.4us of
        # sustained matmul activity; burn the framework preamble + first DMA
        # wait on dummies so the real stream starts warm.
        dums = const.tile([128, 384], bf16)
        nc.gpsimd.memset(dums[:], 0.0)
        warmps = tpsum.tile([128, nb], f32, tag="p2f", name="warmps")
        for wi in range(NWARM):
            nc.tensor.matmul(
                warmps[:, :nb], dums[:, 0:128], dums[:, 128 : 128 + nb],
                start=True, stop=True,
            )

        # --- constants / weights -------------------------------------------
        # sync ring: conv stationaries first, then a small first input chunk
        # so conv can start ~10.5us in, then the 12 input segments.
        t8s = const.tile([128, 8 * KBS], bf16)
        nc.sync.dma_start(t8s[:], t8_d[:])
        xfirst = const.tile([128, nb], bf16)
        nc.sync.dma_start(xfirst[:], xg_d[:, 0, 0, 0:nb])
        xseg = []
        for i in range(NSEG):
            jb, g = i // G, i % G
            xt = const.tile([128, npc], bf16, name=f"xs{i}")
            nc.sync.dma_start(xt[:], xg_d[:, jb, g, :])
            xseg.append(xt)
        # scalar ring: tiny tensors only.
        b1t = const.tile([MTS, MT], f32)
        nc.scalar.dma_start(b1t[:], b1_d[:])
        w2t = const.tile([MTS, MT * FC2_OUT], bf16)
        nc.scalar.dma_start(w2t[:], w2_d[:])
        b2t = const.tile([FC2_OUT, 1], f32)
        nc.scalar.dma_start(b2t[:], b2_d[:])
        # gpsimd (SWDGE) ring: the fc1 weight stream, 6 chunks of 4 blocks.
        w1t = []
        for jc in range(6):
            wt = const.tile([KBS, 4 * FC1_OUT], bf16, name=f"w1c{jc}")
            nc.gpsimd.dma_start(
                wt[:], w1_d[:, jc * 4 * FC1_OUT : (jc + 1) * 4 * FC1_OUT]
            )
            w1t.append(wt)

        ones10 = const.tile([FC2_OUT, FC2_OUT], bf16)
        nc.gpsimd.memset(ones10[:], 1.0)
        # warm the scalar activation table set (exp+relu+copy) while idle
        wout = const.tile([1, 8], f32)
        nc.scalar.activation(wout[:], dums[:1, :8], mybir.ActivationFunctionType.Exp)

        outbuf = const.tile([FC2_OUT, npc], f32)

        def w1_slice(j, mt):
            c0 = (j % 4) * FC1_OUT + mt * MTS
            return w1t[j // 4][:, c0 : c0 + MTS]

        def fc1_mms(fp, a1, j):
            # fp = (fpA, fpB): two 1-bank tiles, each packing 2 M-tiles.
            # start/stop are bank-wide (has_written clears the whole bank),
            # so only the first/last matmul touching a bank carries the flag.
            for mt in range(MT):
                nc.tensor.matmul(
                    fp[mt // 2][:, mt % 2, :],
                    w1_slice(j, mt),
                    a1[j][:],
                    start=(j == 0 and mt % 2 == 0),
                    stop=(j == KB - 1 and mt % 2 == 1),
                )

        def tail_slots(bt, fp, b0):
            """relu -> fc2 -> softmax for batch tile bt, split into slots
            emitted across the first sets of the next batch tile.  The relu
            slots (0,1) must be emitted before the next tile's fc1 j=0 (the
            fpA/fpB slots are reused, bufs=1)."""
            a2t = [None] * MT
            st8 = {}

            def relu_pair(k):
                def f():
                    for mt in (2 * k, 2 * k + 1):
                        a2 = a2pool.tile(
                            [MTS, nb], bf16, tag="a2", name=f"a2_{bt}_{mt}"
                        )
                        src = fp[mt // 2][:, mt % 2, :]
                        if mt % 2 == 0:
                            nc.scalar.activation(
                                a2[:], src, mybir.ActivationFunctionType.Relu,
                                bias=b1t[:, mt : mt + 1],
                            )
                        else:
                            nc.vector.tensor_scalar(
                                a2[:], src, b1t[:, mt : mt + 1], 0.0,
                                mybir.AluOpType.add, mybir.AluOpType.max,
                            )
                        a2t[mt] = a2
                return f

            def fc2():
                p2f = tpsum.tile([FC2_OUT, nb], f32, tag="p2f", name=f"p2f_{bt}")
                for mt in range(MT):
                    nc.tensor.matmul(
                        p2f[:],
                        w2t[:, mt * FC2_OUT : (mt + 1) * FC2_OUT],
                        a2t[mt][:],
                        start=(mt == 0),
                        stop=(mt == MT - 1),
                    )
                st8["p2f"] = p2f

            def expsum():
                e = smpool.tile([FC2_OUT, nb], bf16, tag="e", name=f"e_{bt}")
                nc.scalar.activation(
                    e[:], st8["p2f"][:], mybir.ActivationFunctionType.Exp,
                    bias=b2t[:, 0:1],
                )
                ssum = tpsum.tile([FC2_OUT, nb], f32, tag="ssum", name=f"ss_{bt}")
                nc.tensor.matmul(ssum[:], ones10[:], e[:], start=True, stop=True)
                st8["e"], st8["ssum"] = e, ssum

            def norm():
                rinv = smpool.tile([FC2_OUT, nb], f32, tag="ri", name=f"ri_{bt}")
                nc.vector.reciprocal_approx_fast(rinv[:], st8["ssum"][:])
                nc.vector.tensor_mul(outbuf[:, b0 : b0 + nb], st8["e"][:], rinv[:])
                nc.sync.dma_start(o_d[:, b0 : b0 + nb], outbuf[:, b0 : b0 + nb])

            return [relu_pair(0), relu_pair(1), fc2, expsum, norm]

        pending = []
        for bt in range(nbt):
            b0 = bt * nb
            a1 = [None] * KB
            fp = (
                fpsum.tile([MTS, 2, nb], f32, tag="fpA", name=f"fpA{bt}"),
                fpsum.tile([MTS, 2, nb], f32, tag="fpB", name=f"fpB{bt}"),
            )
            for idx in range(NSEG):
                for st in range(2):
                    sidx = idx * 2 + st
                    if bt == 0 and sidx == 0:
                        rhs = xfirst[:]
                    else:
                        rhs = xseg[idx][:, b0 : b0 + nb]
                    # conv: 4 matmuls -> 2 one-bank PSUM tiles [120, 2, nb]
                    # t0 = conv row 4g+2st (both col parities), t1 = row +1
                    t0 = cpsum.tile([KBS, 2, nb], f32, tag="cps", name=f"t0_{bt}_{sidx}")
                    t1 = cpsum.tile([KBS, 2, nb], f32, tag="cps", name=f"t1_{bt}_{sidx}")
                    for half, tt in ((0, t0), (1, t1)):
                        dd = 2 * st + half
                        for eo in range(2):
                            nc.tensor.matmul(
                                tt[:, eo, :],
                                t8s[:, (dd * 2 + eo) * KBS : (dd * 2 + eo + 1) * KBS],
                                rhs,
                                start=(eo == 0),
                                stop=(eo == 1),
                            )
                    # 2x2 maxpool: max over (t0, t1) x (eo halves).  PSUM
                    # evacuation is split between scalar and DVE; every 6th
                    # set uses 2 scalar copies + a cheap bf16 DVE max instead
                    # of 1 copy + a slow fp32-PSUM DVE max, balancing the
                    # two engines.
                    m = mpool.tile([KBS, 2, nb], bf16, tag="m")
                    if sidx % 6 == 5:
                        s0 = spool.tile([KBS, 2, nb], bf16, tag="s")
                        nc.scalar.copy(s0[:], t0[:])
                        s1 = spool.tile([KBS, 2, nb], bf16, tag="s")
                        nc.scalar.copy(s1[:], t1[:])
                        nc.vector.tensor_max(m[:], s0[:], s1[:])
                    else:
                        s = spool.tile([KBS, 2, nb], f32, tag="sf")
                        nc.scalar.copy(s[:], t0[:])
                        nc.vector.tensor_max(m[:], s[:], t1[:])
                    ab = a1pool.tile([KBS, nb], bf16, tag="a1")
                    nc.vector.tensor_max(ab[:], m[:, 0, :], m[:, 1, :])
                    a1[sidx] = ab
                    # fc1 trails conv so the PE alternates 4 conv + 4 fc1
                    if sidx >= LAG:
                        fc1_mms(fp, a1, sidx - LAG)
                    # previous tile's relu/fc2/softmax, staggered over sets
                    # 1..5 (relu slots land before this tile's fc1 j=0)
                    if pending and 1 <= sidx <= len(pending):
                        pending[sidx - 1]()
                        if sidx == len(pending):
                            pending = []
            for j in range(KB - LAG, KB):
                fc1_mms(fp, a1, j)
            pending = tail_slots(bt, fp, b0)
        for f in pending:
            f()

    nc.compile()
    return nc


def _prep_weights(conv_w, conv_b, fc1_w, fc1_b, fc2_w, fc2_b):
    import ml_dtypes

    bf16 = ml_dtypes.bfloat16
    conv_w = np.asarray(conv_w, np.float32).reshape(COUT, KS, KS)
    conv_b = np.asarray(conv_b, np.float32)
    fc1_w = np.asarray(fc1_w, np.float32)
    fc1_b = np.asarray(fc1_b, np.float32)
    fc2_w = np.asarray(fc2_w, np.float32)
    fc2_b = np.asarray(fc2_b, np.float32)

    # conv stationaries: T8[p = di8*16+jjp, k = dd*2+eo, m = c*6+q]
    #  = conv_w[c, di8-dd, jjp-(2q+eo)] where both offsets are in [0, 5)
    T8 = np.zeros((128, 8, KBS), np.float32)
    for dd in range(4):
        for eo in range(2):
            for c in range(COUT):
                for q in range(6):
                    jj = 2 * q + eo
                    for di in range(KS):
                        for dj in range(KS):
                            T8[(dd + di) * 16 + jj + dj, dd * 2 + eo, c * 6 + q] = (
                                conv_w[c, di, dj]
                            )
    t8 = np.ascontiguousarray(T8.reshape(128, 8 * KBS)).astype(bf16)

    # fc1 weights to pooled-feature order: block j=(jb*6+g)*2+st, row c*6+q
    # -> original flat feature c*144 + (2g+st)*12 + jb*6 + q
    jv = np.arange(KB)
    jbv, gv, stv = jv // 12, (jv % 12) // 2, jv % 2
    ipv = 2 * gv + stv
    ml = np.arange(KBS)
    cv, qv = ml // 6, ml % 6
    fidx = (
        cv[None, :] * 144 + ipv[:, None] * 12 + jbv[:, None] * 6 + qv[None, :]
    )  # [KB, KBS]
    w1 = fc1_w[:, fidx]                       # [500, KB, KBS]
    w1 = np.ascontiguousarray(w1.transpose(2, 1, 0)).reshape(KBS, KB * FC1_OUT)
    w1 = w1.astype(bf16)

    # conv bias folded into fc1 bias (maxpool commutes with per-channel const)
    cb_vec = np.repeat(conv_b, 144)
    b1p = fc1_b + fc1_w @ cb_vec
    b1 = np.ascontiguousarray(b1p.reshape(MT, MTS).T)

    w2 = np.ascontiguousarray(
        fc2_w.T.reshape(MT, MTS, FC2_OUT).transpose(1, 0, 2)
    ).reshape(MTS, MT * FC2_OUT).astype(bf16)
    b2 = np.ascontiguousarray(fc2_b.reshape(FC2_OUT, 1))
    return t8, w1, b1, w2, b2


# im2col pixel indices: IDX[p, jb, g] = (4g + p//16)*28 + 12jb + (p%16)
_P = np.arange(128)
_IDX = (
    (4 * np.arange(G)[None, None, :] + (_P // 16)[:, None, None]) * W
    + 12 * np.arange(JB)[None, :, None]
    + (_P % 16)[:, None, None]
)  # [128, JB, G]


def _prep_x(x_bf16):
    """x_bf16 [npc, 784] bf16 -> xg [128, JB, G, npc] bf16."""
    g = x_bf16[:, _IDX.reshape(-1)]           # [npc, 128*JB*G]
    g = g.T.reshape(128, JB, G, x_bf16.shape[0])
    return np.ascontiguousarray(g)


def _run(inputs, npc=NPC, nb=256, trace=False):
    import ml_dtypes
    from concourse import bass_utils

    key = (npc, nb)
    if key not in _cache:
        _cache[key] = _build(npc, nb)
    nc = _cache[key]

    t8, w1, b1, w2, b2 = _prep_weights(
        inputs["conv_w"], inputs["conv_b"], inputs["fc1_w"],
        inputs["fc1_b"], inputs["fc2_w"], inputs["fc2_b"],
    )
    x = np.asarray(inputs["x"], np.float32).reshape(-1, H * W)
    n_total = x.shape[0]
    assert n_total == NCORES * npc
    xb = x.astype(ml_dtypes.bfloat16)

    in_maps = [
        {
            "xg": _prep_x(xb[i * npc : (i + 1) * npc]),
            "t8": t8, "w1": w1, "b1": b1, "w2": w2, "b2": b2,
        }
        for i in range(NCORES)
    ]
    res = bass_utils.run_bass_kernel_spmd(
        nc, in_maps, core_ids=list(range(NCORES)), trace=trace
    )
    out = np.concatenate(
        [np.asarray(res.results[i]["out"], np.float32).T for i in range(NCORES)],
        axis=0,
    )
    return out, res


def kernel(**inputs):
    out, _ = _run(inputs)
    return out
